# revision 8
# baseline (speedup 1.0000x reference)
"""Causal self-attention on 8 TRN2 NeuronCores — bf16 redesign.

Sharding: data-parallel over batch (2) x tensor-parallel over heads (4 heads
per core). Core c handles batch c//4, heads 4*(c%4)..4*(c%4)+3 — i.e. columns
[256*g, 256*(g+1)) of wq/wk/wv and rows [256*g, 256*(g+1)) of wo. Each core
returns a partial output [2048, 1024]; the host sums the 4 partials of each
batch and adds the (bv @ wo + bo) correction (exact because softmax rows sum
to 1).

Per-core kernel (Tile framework, fully unrolled, bf16 storage / f32 psum):
  1. Host pre-transposes x, so xT [1024, 2048] chunks DMA straight into SBUF
     (no PE transposes). qT/kT [256,2048] projected with xT as the moving
     operand (j on partitions; q scaled by 1/8 + bq, k + bk fused into the
     psum->sbuf move). v projected in natural [t, j] layout (xT chunk as the
     stationary) straight into v_aug, which carries a ones column per head
     ([128, 65] groups) so the AV matmul also produces softmax denominators.
  2. Scores per (head-pair, 512-wide i-block) kept TRANSPOSED ([l-chunk=128,
     i=512]); the two heads of a pair occupy disjoint PE row groups (K=64 at
     rows 0-63/64-127). One [128,<=1024] exp covers both heads. Causal:
     chunks above the diagonal are skipped; diagonal chunks compute only the
     live column range and are masked by ACCUMULATING a strictly-triangular
     -60 tile onto the scores via a tiny PE matmul (exp underflows to 0) —
     no cross-engine mask op on the chunk critical path.
  3. AV with the QUERY dim on output partitions: out[i-tile 128, 65] =
     ex-slice.T @ v_aug — full 128 output partitions per cycle (2x the
     head-on-partitions layout) and the denominator lands as a per-partition
     column, so normalization is a reciprocal + per-partition tensor_scalar
     (no partition_broadcast). Each (head, i-tile) PSUM accumulation window
     is emitted CONTIGUOUSLY (interleaved windows within one PSUM bank lose
     their first write on hardware), one diagonal chunk behind the exps it
     consumes.
  4. Attention output tiles [128 t, 256 j] are PE-transposed back to [j, t]
     for the out-projection; y is written bf16 (the host sums partials in
     f32).
  5. Schedule: software-pipelined — next block's projections interleave with
     the current block's ACT(exp)-paced attention chunks, weighted by PE
     starvation; all deferred out-projections park in the last (most
     exp-bound) block. PSUM = 2x[128,1024] scores + 2x[128,260] AV +
     2x[128,512] fillers = 8 banks.
"""

import sys

import numpy as np

if "/opt/trn_rl_repo" not in sys.path:
    sys.path.insert(0, "/opt/trn_rl_repo")

import concourse.mybir as mybir
import concourse.tile as tile
from concourse import bacc
from concourse.bass_utils import run_bass_kernel_spmd

# Problem shapes (hardcoded per contract)
B, S, D = 2, 2048, 1024
H, DH = 16, 64
NCORES = 8
GROUPS = 4                  # tensor-parallel groups per batch
HL = H // GROUPS            # 4 local heads
JC = HL * DH                # 256 local head columns
T = S                       # tokens per core (one batch element)

P = 128                     # partitions
TS = 512                    # token block (projection granularity)
NTB = T // TS               # 4 token blocks
NDC = D // P                # 8 contraction chunks
IB = 512                    # attention i-block (query positions)
LCH = P                     # attention l-chunk (key positions)
VA = DH + 1                 # v_aug columns per head (ones column appended)
NIT = IB // P               # 4 i-tiles per block

FP = mybir.dt.float32
BF = mybir.dt.bfloat16
F8 = mybir.dt.float8e4

_CACHE = {}


def build_nc():
    nc = bacc.Bacc("TRN2", target_bir_lowering=False, debug=False)

    import os
    qk_fp8 = os.environ.get("K_QK_FP8", "1") == "1"
    QKDT = F8 if qk_fp8 else BF
    xt_d = nc.dram_tensor("xt", [D, T], BF, kind="ExternalInput")
    if qk_fp8:
        xt8_d = nc.dram_tensor("xt8", [D, T], F8, kind="ExternalInput")
    wq = nc.dram_tensor("wq", [D, JC], QKDT, kind="ExternalInput")
    wk = nc.dram_tensor("wk", [D, JC], QKDT, kind="ExternalInput")
    wv = nc.dram_tensor("wv", [D, JC], BF, kind="ExternalInput")
    wo = nc.dram_tensor("wo", [JC, D], BF, kind="ExternalInput")
    bq = nc.dram_tensor("bq", [JC, 1], FP, kind="ExternalInput")
    bk = nc.dram_tensor("bk", [JC, 1], FP, kind="ExternalInput")
    y = nc.dram_tensor("y", [T, D], BF, kind="ExternalOutput")
    dbg = os.environ.get("BASSDBG") == "1"
    # schedule-tuning knobs (default = swept best; env for experiments)
    exp_dve_mod = int(os.environ.get("K_EXP_DVE_MOD", "-3"))  # 0 = off
    ycopy_act = os.environ.get("K_YCOPY_ACT", "0") == "1"
    diag_deep = os.environ.get("K_DIAG_DEEP", "1") == "1"
    n_warm = int(os.environ.get("K_WARM", "56"))
    w_full = float(os.environ.get("K_W_FULL", "3.0"))
    w_diag = float(os.environ.get("K_W_DIAG", "1.0"))
    if dbg:
        dbg_ao = nc.dram_tensor("dbg_ao", [P, (T // P) * HL * DH], BF, kind="ExternalOutput")
        dbg_qt = nc.dram_tensor("dbg_qt", [P, T], BF, kind="ExternalOutput")
        dbg_kt = nc.dram_tensor("dbg_kt", [P, T], BF, kind="ExternalOutput")
        dbg_va = nc.dram_tensor("dbg_va", [P, (T // P) * HL * VA], BF, kind="ExternalOutput")
        dbg_av = nc.dram_tensor("dbg_av", [P, NIT * VA], FP, kind="ExternalOutput")
        dbg_ex = nc.dram_tensor("dbg_ex", [P, 2 * IB], BF, kind="ExternalOutput")

    with tile.TileContext(nc) as tc:
        import contextlib

        with contextlib.ExitStack() as ctx:
            singles = ctx.enter_context(tc.tile_pool(name="singles", bufs=1))
            xt_pool = ctx.enter_context(tc.tile_pool(name="xt", bufs=2))
            # all of an i-block pair's exp tiles stay live until its AV pass
            # (16 chunks at i=3); block 3's pulled-forward full chunks add to
            # the peak while block 2's close phase is still draining
            exp_pool = ctx.enter_context(tc.tile_pool(name="exp", bufs=32))
            nrm_pool = ctx.enter_context(tc.tile_pool(name="nrm", bufs=4))
            ysb_pool = ctx.enter_context(tc.tile_pool(name="ysb", bufs=4))
            # PSUM: tag "big" 2x[128,1024] f32 (qT/kT proj pairs, then score
            # pairs), "av" 2x[128,260] f32 (AV + denominators, one per head),
            # "fil" 2x[128,512] (proj, aoT transposes, out-proj) = 8 banks.
            ps = ctx.enter_context(tc.tile_pool(name="ps", bufs=2, space="PSUM"))

            def load_block(tb, halves=1):
                xt = xt_pool.tile([P, NDC * TS], BF, tag="xt", name=f"xt{tb}")
                xt_g = xt.rearrange("p (c t) -> p c t", t=TS)
                src = xt_d[:, :].rearrange("(c p) t -> p c t", p=P)[
                    :, :, TS * tb:TS * (tb + 1)]
                h = NDC // halves
                xt8 = None
                if qk_fp8:
                    # all fp8 halves first: the q/k projections (which lead
                    # the filler stream) only need xt8; bf16 xt is for the
                    # trailing v-projection units
                    xt8 = xt_pool.tile([P, NDC * TS], F8, tag="xt8", name=f"xt8_{tb}")
                    src8 = xt8_d[:, :].rearrange("(c p) t -> p c t", p=P)[
                        :, :, TS * tb:TS * (tb + 1)]
                    for k in range(halves):
                        nc.sync.dma_start(
                            out=xt8.rearrange("p (c t) -> p c t", t=TS)[
                                :, h * k:h * (k + 1), :],
                            in_=src8[:, h * k:h * (k + 1), :],
                        )
                for k in range(halves):
                    nc.sync.dma_start(
                        out=xt_g[:, h * k:h * (k + 1), :],
                        in_=src[:, h * k:h * (k + 1), :],
                    )
                return xt, xt8

            # ---- DMA order = need order: wq/bq + first xt half unblock the
            # first projection ~2.5us in; wo is not needed until ~40us ----
            wq_sb = singles.tile([P, NDC * JC], QKDT, tag="wq")  # chunk c at [JC*c, JC*(c+1))
            wk_sb = singles.tile([P, NDC * JC], QKDT, tag="wk")
            wv_sb = singles.tile([P, NDC * JC], BF, tag="wv")
            bq_sb = singles.tile([P, 2], FP, tag="bq")
            bk_sb = singles.tile([P, 2], FP, tag="bk")
            wo_sb = singles.tile([P, 2 * D], BF, tag="wo")      # j-chunk jc at [D*jc, ...)
            nc.sync.dma_start(
                out=wq_sb.rearrange("p (c j) -> p c j", j=JC),
                in_=wq[:, :].rearrange("(c p) j -> p c j", p=P),
            )
            nc.sync.dma_start(
                out=bq_sb, in_=bq[:, :].rearrange("(j p) one -> p (j one)", p=P),
            )
            nc.sync.dma_start(
                out=wk_sb.rearrange("p (c j) -> p c j", j=JC),
                in_=wk[:, :].rearrange("(c p) j -> p c j", p=P),
            )
            nc.sync.dma_start(
                out=bk_sb, in_=bk[:, :].rearrange("(j p) one -> p (j one)", p=P),
            )
            xt0 = load_block(0, halves=2)
            nc.sync.dma_start(
                out=wv_sb.rearrange("p (c j) -> p c j", j=JC),
                in_=wv[:, :].rearrange("(c p) j -> p c j", p=P),
            )
            nc.sync.dma_start(
                out=wo_sb.rearrange("p (c d) -> p c d", d=D),
                in_=wo[:, :].rearrange("(c p) d -> p c d", p=P),
            )

            # bf16 identity for the ao PE transposes (memset works for bf16,
            # so no dependency on any DMA — warmup can start immediately)
            ident = singles.tile([P, P], BF)
            nc.vector.memset(ident, 0.0)
            nc.gpsimd.affine_select(
                out=ident, in_=ident, compare_op=mybir.AluOpType.not_equal,
                fill=1.0, base=0, channel_multiplier=1, pattern=[[-1, P]],
            )
            # strictly-upper-triangular -BIG tile (transposed for use as the
            # stationary of the causal-mask matmul: TRIT.T @ I adds -BIG
            # where key > query, so exp() underflows to 0 -- replaces the
            # per-chunk gpsimd affine_select and its cross-engine sync)
            trit = singles.tile([P, P], BF)
            nc.vector.memset(trit, 0.0)
            nc.gpsimd.affine_select(
                out=trit, in_=trit, compare_op=mybir.AluOpType.is_ge,
                fill=-60.0, base=0, channel_multiplier=1, pattern=[[-1, P]],
            )

            # persistent activations
            qt_sb = [singles.tile([P, T], BF, tag=f"qt{j}", name=f"qt_sb{j}") for j in range(2)]
            kt_sb = [singles.tile([P, T], BF, tag=f"kt{j}", name=f"kt_sb{j}") for j in range(2)]
            # attention out, natural layout: tile tt holds [128 t, 4 heads x 64]
            ao_nat = singles.tile([P, (T // P) * HL * DH], BF, tag="aon")
            # attention out transposed: j-chunk jc at cols [T*jc, T*(jc+1))
            aoT_sb = singles.tile([P, 2 * T], BF, tag="aot")
            # v_aug: l-chunk lc at [VA*HL*lc, ...), head h at offset VA*h, ones at +DH
            n_lch = T // LCH
            vaug = singles.tile([P, n_lch * HL * VA], BF, tag="vaug")
            vaug_g = vaug.rearrange("p (c v) -> p c v", v=VA)
            nc.vector.memset(vaug_g[:, :, DH], 1.0)   # ones columns

            # PE warm-up: dummy matmuls during the initial DMA wait get the
            # HAM clock gate to full rate before the real work arrives.
            warm = ps.tile([P, P], FP, tag="fil", name="warm")
            # first warms only need a DVE memset (no Pool affine hop), so the
            # PE ramp starts a few hundred ns before ident is ready
            junk = singles.tile([P, P], BF, tag="junk")
            nc.vector.memset(junk, 0.0)
            for _ in range(8):
                nc.tensor.matmul(warm, junk, junk, start=True, stop=True)
            for _ in range(n_warm - 8):
                nc.tensor.matmul(warm, ident, ident, start=True, stop=True)

            # ---------- emission units (software-pipelined schedule) ----------
            def proj_units(tb, xt, xt8):
                """Single-bank filler units: q/k transposed per j-tile, v in
                natural [token, head-col] layout straight into v_aug."""
                units = []
                qk_src = xt8 if qk_fp8 else xt

                def qk_mm(acc, w_sb, j, c, start, stop):
                    if qk_fp8:
                        # fp8 DoubleRow: chunk PAIR (2c, 2c+1) packed as
                        # k-tiles — both operands use dims [p, ktile, free],
                        # halving the per-output-row cost
                        w_g = w_sb.rearrange("p (c j) -> p c j", j=JC)
                        x_g = qk_src.rearrange("p (c t) -> p c t", t=TS)
                        nc.tensor.matmul(
                            acc,
                            w_g[:, 2 * c:2 * c + 2, P * j:P * (j + 1)],
                            x_g[:, 2 * c:2 * c + 2, :],
                            start=start, stop=stop,
                            perf_mode=mybir.MatmulPerfMode.DoubleRow,
                        )
                    else:
                        nc.tensor.matmul(
                            acc,
                            w_sb[:, JC * c + P * j:JC * c + P * (j + 1)],
                            qk_src[:, TS * c:TS * (c + 1)],
                            start=start, stop=stop,
                        )

                def make_qk(which, w_sb, out_sb, j):
                    box = [None]
                    nqk = NDC // 2 if qk_fp8 else NDC   # chunk-pairs vs chunks

                    def emit_lo():
                        box[0] = ps.tile([P, TS], FP, tag="fil", name=f"{which}p{tb}_{j}")
                        for c in range(nqk // 2):
                            qk_mm(box[0], w_sb, j, c, start=(c == 0), stop=False)

                    def emit_hi():
                        acc = box[0]
                        for c in range(nqk // 2, nqk):
                            qk_mm(acc, w_sb, j, c, start=False, stop=(c == nqk - 1))
                        if which == "qt":
                            nc.vector.tensor_scalar(
                                out=out_sb[j][:, TS * tb:TS * (tb + 1)], in0=acc,
                                scalar1=0.125, scalar2=bq_sb[:, j:j + 1],
                                op0=mybir.AluOpType.mult, op1=mybir.AluOpType.add,
                            )
                        else:
                            nc.vector.tensor_scalar(
                                out=out_sb[j][:, TS * tb:TS * (tb + 1)], in0=acc,
                                scalar1=bk_sb[:, j:j + 1], scalar2=None,
                                op0=mybir.AluOpType.add,
                            )
                    return [emit_lo, emit_hi]

                def make_v(s):
                    box = [None]

                    def make_w(w):
                        def emit():
                            # natural [t, j] layout (xT chunk is the stationary);
                            # sequential accumulation groups per bank half
                            if w == 0:
                                box[0] = ps.tile([P, TS], FP, tag="fil", name=f"vp{tb}_{s}")
                            acc = box[0]
                            ts_ = 2 * s + w
                            for c in range(NDC):
                                nc.tensor.matmul(
                                    acc[:, JC * w:JC * (w + 1)],
                                    xt[:, TS * c + P * ts_:TS * c + P * (ts_ + 1)],
                                    wv_sb[:, JC * c:JC * (c + 1)],
                                    start=(c == 0), stop=(c == NDC - 1),
                                )
                            lc = 4 * tb + ts_
                            nc.vector.tensor_copy(
                                out=vaug_g[:, HL * lc:HL * (lc + 1), 0:DH],
                                in_=acc[:, JC * w:JC * (w + 1)].rearrange(
                                    "p (h d) -> p h d", d=DH
                                ),
                            )
                        return emit
                    return [make_w(0), make_w(1)]

                for j in range(2):
                    units.extend(make_qk("qt", wq_sb, qt_sb, j))
                    units.extend(make_qk("kt", wk_sb, kt_sb, j))
                for s in range(2):
                    units.extend(make_v(s))
                return units

            def attn_units(i):
                """Returns (full_units, full_w, close_units, close_w):
                the full-chunk stretch is ACT(exp)-paced with little PE work
                (weight ~ starvation); the close stretch (diagonal chunks +
                AV windows + tails) is PE-rich."""
                nch = 4 * (i + 1)   # causal chunks
                units = []
                weights = []
                cunits = []
                cweights = []
                for jp in range(2):          # head pair (2*jp, 2*jp+1)
                    avs = [None, None]
                    exs = [None] * nch       # per-chunk exp tiles (kept in SBUF)

                    def make_pair_start(i, jp, avs):
                        def emit():
                            for u in range(2):
                                avs[u] = ps.tile(
                                    [P, NIT * VA], FP, tag="av", name=f"av{i}_{2 * jp + u}"
                                )
                        return emit

                    def make_c(i, jp, exs, c, dve_exp=False):
                        def emit():
                            # Diagonal chunks only need query columns >= 128*v
                            # (earlier ones are fully masked).
                            diag = c >= 4 * i
                            v = c - 4 * i if diag else 0
                            off = P * v
                            ne = IB - off
                            # both heads' scoresT for chunk c in one 2-bank tile
                            sc = ps.tile([P, 2 * IB], FP, tag="big", name=f"sc{i}_{jp}_{c}")
                            for u in range(2):
                                ro = DH * u
                                nc.tensor.matmul(
                                    sc[:, IB * u + off:IB * (u + 1)],
                                    kt_sb[jp][ro:ro + DH, LCH * c:LCH * (c + 1)],
                                    qt_sb[jp][ro:ro + DH, IB * i + off:IB * (i + 1)],
                                    start=True, stop=not diag,
                                )
                                if diag:
                                    # causal mask: add -60 where key > query in
                                    # the triangular i-tile, so exp() gives ~0.
                                    # PE-local: no cross-engine sync needed.
                                    nc.tensor.matmul(
                                        sc[:, IB * u + off:IB * u + off + P],
                                        trit, ident,
                                        start=False, stop=True,
                                        skip_group_check=True,
                                    )
                            ex = exp_pool.tile([P, 2 * IB], BF, tag="ex", name=f"ex{i}_{jp}_{c}")
                            exs[c] = ex
                            sc_g = sc.rearrange("p (u n) -> p u n", u=2)
                            ex_g = ex.rearrange("p (u n) -> p u n", u=2)
                            if dve_exp:
                                # bit-trick exp on DVE (full chunks only):
                                # bf16 bits of e^x are ~ round(x*128/ln2 +
                                # magic); +-3% weight noise, washes out in
                                # softmax. Offloads the exp-bound late blocks.
                                nc.vector.tensor_scalar(
                                    out=ex.bitcast(mybir.dt.int16), in0=sc,
                                    scalar1=184.66500816464, scalar2=16248.6,
                                    op0=mybir.AluOpType.mult,
                                    op1=mybir.AluOpType.add,
                                )
                            else:
                                nc.scalar.activation(
                                    out=ex_g[:, :, off:], in_=sc_g[:, :, off:],
                                    func=mybir.ActivationFunctionType.Exp,
                                )
                            if dbg and i == 0 and jp == 0 and c == 0:
                                nc.sync.dma_start(out=dbg_ex[:, :], in_=ex)
                        return emit

                    def make_av(i, jp, avs, exs, u, it):
                        # one contiguous PSUM accumulation window per (head,
                        # i-tile): interleaved windows within a bank lose
                        # their first write on hardware.
                        def emit():
                            h = 2 * jp + u
                            for c in range(4 * i + it + 1):
                                nc.tensor.matmul(
                                    avs[u][:, VA * it:VA * (it + 1)],
                                    exs[c][:, IB * u + P * it:IB * u + P * (it + 1)],
                                    vaug[:, VA * (HL * c + h):VA * (HL * c + h + 1)],
                                    start=(c == 0), stop=(c == 4 * i + it),
                                    skip_group_check=True,
                                )
                        return emit

                    def make_tail(i, jp, avs, u):
                        def emit():
                            h = 2 * jp + u
                            av_g = avs[u].rearrange("p (t v) -> p t v", v=VA)
                            if dbg and i == 0 and jp == 0 and u == 0:
                                av_dbg = singles.tile([P, NIT * VA], FP, tag="avdbg")
                                nc.vector.tensor_copy(out=av_dbg, in_=avs[u])
                                nc.sync.dma_start(out=dbg_av[:, :], in_=av_dbg)
                            rc = nrm_pool.tile([P, NIT], FP, tag="rc", name=f"rc{i}_{h}")
                            nc.vector.reciprocal(out=rc, in_=av_g[:, :, DH])
                            for it in range(NIT):
                                tt = NIT * i + it
                                nc.vector.tensor_scalar(
                                    out=ao_nat[:, JC * tt + P * jp + DH * u:
                                               JC * tt + P * jp + DH * (u + 1)],
                                    in0=av_g[:, it, 0:DH],
                                    scalar1=rc[:, it:it + 1], scalar2=None,
                                    op0=mybir.AluOpType.mult,
                                )
                        return emit

                    units, weights = cunits, cweights   # per-jp: full then close
                    for c in range(4 * i):
                        # offload some exps to DVE in the exp-bound late
                        # blocks (DVE also carries copies + normalize there);
                        # negative mod = last block only (its chunk phase is
                        # ACT-paced with DVE mostly idle)
                        if exp_dve_mod > 0:
                            dve = i >= 2 and c % exp_dve_mod == 2 % exp_dve_mod
                        elif exp_dve_mod < 0:
                            dve = i == 3 and c % -exp_dve_mod == 2 % -exp_dve_mod
                        else:
                            dve = False
                        units.append(make_c(i, jp, exs, c, dve_exp=dve))
                        weights.append(w_full)
                    cunits.append(make_pair_start(i, jp, avs))
                    cweights.append(0.0)
                    # AV window (u, it) completes with diagonal chunk 4i+it:
                    # emit it one diag chunk later so its tail never waits on
                    # the freshest exp
                    if diag_deep:
                        for v in range(NIT):
                            cunits.append(make_c(i, jp, exs, 4 * i + v))
                            cweights.append(w_diag)
                            if v >= 1:
                                for u in range(2):
                                    cunits.append(make_av(i, jp, avs, exs, u, v - 1))
                                    cweights.append(0.0)
                        for u in range(2):
                            cunits.append(make_av(i, jp, avs, exs, u, NIT - 1))
                            cweights.append(0.0)
                    else:
                        for v in range(NIT):
                            cunits.append(make_c(i, jp, exs, 4 * i + v))
                            cweights.append(w_diag)
                            for u in range(2):
                                cunits.append(make_av(i, jp, avs, exs, u, v))
                                cweights.append(0.0)
                    cunits.append(make_tail(i, jp, avs, 0))
                    cweights.append(0.0)
                    cunits.append(make_tail(i, jp, avs, 1))
                    cweights.append(0.0)
                return cunits, cweights

            def y_units(i):
                units = []

                def make(tt):
                    ysb_box = [None]

                    def emit_tr():
                        trp = ps.tile([P, 2 * P], BF, tag="fil", name=f"tr{tt}")
                        for jc in range(2):
                            nc.tensor.transpose(
                                trp[:, P * jc:P * (jc + 1)],
                                ao_nat[:, JC * tt + P * jc:JC * tt + P * (jc + 1)],
                                ident,
                            )
                        # one copy moves both j-chunks into aoT (jc-major)
                        nc.vector.tensor_copy(
                            out=aoT_sb.rearrange("p (j t) -> p j t", j=2)[
                                :, :, P * tt:P * (tt + 1)],
                            in_=trp.rearrange("p (j t) -> p j t", j=2),
                        )

                    def make_yp(db):
                        def emit():
                            if db == 0:
                                ysb_box[0] = ysb_pool.tile(
                                    [P, D], BF, tag="ysb", name=f"ysb{tt}"
                                )
                            ysb = ysb_box[0]
                            yps = ps.tile([P, IB], FP, tag="fil", name=f"yps{tt}_{db}")
                            for jc in range(2):
                                nc.tensor.matmul(
                                    yps,
                                    aoT_sb[:, T * jc + P * tt:P * (tt + 1) + T * jc],
                                    wo_sb[:, D * jc + IB * db:D * jc + IB * (db + 1)],
                                    start=(jc == 0), stop=(jc == 1),
                                )
                            # y copies on DVE; the last block's go to ACT,
                            # which is idle once the final exps are done
                            if (ycopy_act or tt >= 12) and (tt + db) % 2 == 0:
                                nc.scalar.activation(
                                    out=ysb[:, IB * db:IB * (db + 1)], in_=yps,
                                    func=mybir.ActivationFunctionType.Copy,
                                )
                            else:
                                nc.vector.tensor_copy(
                                    out=ysb[:, IB * db:IB * (db + 1)], in_=yps,
                                )
                            # DMA per 512-col half: overlaps the other half's
                            # copy and shortens the end-of-kernel chain
                            nc.sync.dma_start(
                                out=y[P * tt:P * (tt + 1), IB * db:IB * (db + 1)],
                                in_=ysb[:, IB * db:IB * (db + 1)],
                            )
                        return emit
                    return [emit_tr, make_yp(0), make_yp(1)]

                for tt in range(NIT * i, NIT * (i + 1)):
                    units.extend(make(tt))
                return units

            def interleave(main, fillers, weights=None):
                """Emit `main` units with `fillers` spread between them,
                proportionally to per-unit `weights` (PE-starvation demand)."""
                if not main:
                    for f in fillers:
                        f()
                    return
                if weights is None:
                    weights = [1.0] * len(main)
                tot = sum(weights) or 1.0
                nf = len(fillers)
                fi = 0
                acc = 0.0
                for m, w in zip(main, weights):
                    m()
                    acc += w
                    want = int(round(acc / tot * nf))
                    while fi < want:
                        fillers[fi]()
                        fi += 1
                while fi < nf:
                    fillers[fi]()
                    fi += 1

            # ---------- pipelined schedule ----------
            # NOTE: Tile is a *tracing* scheduler — emission order defines the
            # dataflow. Every consumer must be emitted after its producer, so
            # block-0 setup runs as a strict prologue.
            for u in proj_units(0, *xt0):
                u()

            def merge(ua, wa, ub, wb):
                """Proportional round-robin merge of two unit streams."""
                out_u, out_w = [], []
                ia = ib2 = 0
                na, nb = len(ua), len(ub)
                for _ in range(na + nb):
                    if ib2 >= nb or (ia < na and ia * nb <= ib2 * na):
                        out_u.append(ua[ia]); out_w.append(wa[ia]); ia += 1
                    else:
                        out_u.append(ub[ib2]); out_w.append(wb[ib2]); ib2 += 1
                return out_u, out_w

            # blocks 0-2: attention + next block's load/projections
            for tb in range(3):
                cu, cw = attn_units(tb)
                nxt = load_block(tb + 1)
                interleave(cu, proj_units(tb + 1, *nxt), cw)
            # the last block's attention is ACT(exp)-bound with ~8us of PE
            # slack and no next-block projections: park ALL deferred
            # out-projection blocks here as PE filler
            cu, cw = attn_units(3)
            interleave(cu, y_units(0) + y_units(1) + y_units(2), cw)
            for u in y_units(NTB - 1):
                u()
            if dbg:
                nc.sync.dma_start(out=dbg_ao[:, :], in_=ao_nat)
                nc.sync.dma_start(out=dbg_qt[:, :], in_=qt_sb[0])
                nc.sync.dma_start(out=dbg_kt[:, :], in_=kt_sb[0])
                nc.sync.dma_start(out=dbg_va[:, :], in_=vaug)

    nc.compile()
    return nc


def get_nc():
    if "nc" not in _CACHE:
        _CACHE["nc"] = build_nc()
    return _CACHE["nc"]


def kernel(x, wq, bq, wk, bk, wv, bv, wo, bo):
    import os
    import ml_dtypes
    BF_NP = ml_dtypes.bfloat16
    F8_NP = ml_dtypes.float8_e4m3
    qk_fp8 = os.environ.get("K_QK_FP8", "1") == "1"

    x = np.asarray(x, dtype=np.float32)
    wq = np.asarray(wq, dtype=np.float32)
    wk = np.asarray(wk, dtype=np.float32)
    wv = np.asarray(wv, dtype=np.float32)
    wo = np.asarray(wo, dtype=np.float32)
    bq = np.asarray(bq, dtype=np.float32)
    bk = np.asarray(bk, dtype=np.float32)
    bv = np.asarray(bv, dtype=np.float32)
    bo = np.asarray(bo, dtype=np.float32)

    nc = get_nc()
    xt_f32 = [np.ascontiguousarray(x[b].T) for b in range(B)]
    xt_by_batch = [xb.astype(BF_NP) for xb in xt_f32]
    qk_dt = F8_NP if qk_fp8 else BF_NP
    wq_c = wq.astype(qk_dt)
    wk_c = wk.astype(qk_dt)
    wv_bf = wv.astype(BF_NP)
    wo_bf = wo.astype(BF_NP)
    if qk_fp8:
        xt8_by_batch = [xb.astype(F8_NP) for xb in xt_f32]

    in_maps = []
    for core in range(NCORES):
        b, g = divmod(core, GROUPS)
        cs = slice(JC * g, JC * (g + 1))
        im = {
            "xt": xt_by_batch[b],
            "wq": np.ascontiguousarray(wq_c[:, cs]),
            "wk": np.ascontiguousarray(wk_c[:, cs]),
            "wv": np.ascontiguousarray(wv_bf[:, cs]),
            "wo": np.ascontiguousarray(wo_bf[cs, :]),
            "bq": np.ascontiguousarray(bq[cs].reshape(JC, 1)),
            "bk": np.ascontiguousarray(bk[cs].reshape(JC, 1)),
        }
        if qk_fp8:
            im["xt8"] = xt8_by_batch[b]
        in_maps.append(im)
    res = run_bass_kernel_spmd(nc, in_maps, list(range(NCORES)))
    _CACHE["last_results"] = res

    out = np.zeros((B, S, D), np.float32)
    for core in range(NCORES):
        out[core // GROUPS] += res.results[core]["y"]
    # bv and bo never pass through softmax nonlinearity: rows of attn sum to 1,
    # so (v + bv) contributes exactly bv @ wo to every output row.
    out += (bv @ wo + bo)[None, None, :]
    return out
